# revision 1
# baseline (speedup 1.0000x reference)
"""Causal multi-head attention block on 8 TRN2 NeuronCores.

Sharding: tensor-parallel over heads (2 heads/core, both batches) for the
QKV projection + attention; an on-device AllToAll re-shards to
sequence-parallel for the output projection (Megatron-style). Matmuls run in
float32r (full PE rate, ~1.5e-4 rel err). Scores use zero-padded per-head
K^T copies so they run as full K=128 matmuls (half-height K=64 row-tiled
matmuls measured 1.8x slower per-op on HW).

Self-contained: hardcodes all shapes from the problem spec.
"""

import numpy as np
from contextlib import ExitStack

import concourse.bass as bass
import concourse.tile as tile
from concourse import bacc, mybir
from concourse.bass_utils import run_bass_kernel_spmd

F32R = mybir.dt.float32r
F32 = mybir.dt.float32
AF = mybir.ActivationFunctionType

B, T, C, H, HD = 2, 2048, 1024, 16, 64
NCORES = 8
BT = B * T            # 4096 global rows
TQ = 512              # q-chunk width
KT = 128              # k-tile height
NJ = T // TQ          # 4 q-chunks per batch (= per core)
NKK = T // KT         # 16 k-tiles per batch
NCT = C // 128        # 8 contraction tiles for projections
NTC = BT // TQ        # 8 global t-chunks
TSL = BT // NCORES    # 512 rows of final output per core
SPOOL_BUFS = 3
PO_BUFS = 2
QUICK_EVICT = True
ATTN_INTERLEAVE = False
XT_SHAPE = [NCT, NTC, 128, TQ]  # tile-contiguous full x^T


def build(with_collective=True):
    nc = bacc.Bacc(None, target_bir_lowering=False)

    xt = nc.dram_tensor("xt", XT_SHAPE, F32R, kind="ExternalInput")
    wqkv = nc.dram_tensor("wqkv", [C, 3 * 128], F32R, kind="ExternalInput")
    bqkv = nc.dram_tensor("bqkv", [128, 3], F32, kind="ExternalInput")
    wout = nc.dram_tensor("wout", [C, C], F32R, kind="ExternalInput")
    bout = nc.dram_tensor("bout", [128, C], F32, kind="ExternalInput")
    out = nc.dram_tensor("out", [TSL, C], F32, kind="ExternalOutput")

    ident_d = nc.dram_tensor("ident", [128, 128], F32R, kind="ExternalInput")
    ones_d = nc.dram_tensor("ones", [128, 64], F32R, kind="ExternalInput")
    zeros_d = nc.dram_tensor("zeros", [64, TQ], F32R, kind="ExternalInput")
    a2a_in = nc.dram_tensor("a2a_in", [NCORES, 128, TQ], F32R)
    a2a_out = nc.dram_tensor("a2a_out", [NCORES, 128, TQ], F32R)

    with tile.TileContext(nc) as tc:
        _emit(nc, tc, xt, wqkv, bqkv, wout, bout, out, a2a_in, a2a_out,
              ident_d, ones_d, zeros_d, with_collective)
    nc.compile()
    return nc


def _emit(nc, tc, xt, wqkv, bqkv, wout, bout, out, a2a_in, a2a_out,
          ident_d, ones_d, zeros_d, with_collective, prefetch_wout="mid",
          trunc=None):
    with ExitStack() as ctx:
        persist = ctx.enter_context(tc.tile_pool(name="persist", bufs=1))

        # persistent SBUF tensors, indexed by batch b (the core owns the
        # same 2 heads in both batches).
        qts = [persist.tile([128, T], F32R, tag=f"qt{p}", name=f"qt{p}")
               for p in range(2)]
        # zero-padded per-head K^T (head h lives in rows 64*(h%2);
        # the other 64 rows are zero so scores run as full K=128 matmuls)
        kts = [persist.tile([128, T], F32R, tag=f"kt{h}", name=f"kt{h}")
               for h in range(4)]
        va = persist.tile([128, 2, NKK, 192], F32R, tag="va")  # [V_e|ones|V_o]
        wsb = persist.tile([128, NCT, 384], F32R, tag="wsb")
        bsb = persist.tile([128, 3], F32, tag="bsb")
        ident = persist.tile([128, 128], F32R, tag="ident")
        wosb = persist.tile([128, NCT, C], F32R, tag="wo")
        bosb = persist.tile([128, C], F32, tag="bo")

        nc.sync.dma_start(wsb[:], wqkv[:].rearrange("(n p) c -> p n c", p=128))
        nc.sync.dma_start(bsb[:], bqkv[:])
        if prefetch_wout == "early":
            nc.sync.dma_start(wosb[:], wout[:].rearrange("(n p) c -> p n c", p=128))
            nc.sync.dma_start(bosb[:], bout[:])

        # zero padding of kts + identity + VA ones (host constants)
        for h in range(4):
            dead = slice(64, 128) if h % 2 == 0 else slice(0, 64)
            for z in range(NJ):
                nc.sync.dma_start(kts[h][dead, TQ * z:TQ * (z + 1)], zeros_d[:])
        nc.sync.dma_start(ident[:], ident_d[:])
        for p0 in range(2):
            for tt0 in range(NKK):
                nc.sync.dma_start(va[:, p0, tt0, 64:128], ones_d[:])

        # ---- phase 1: QKV^T projection (+ V transpose into VA) ----
        with (
            tc.tile_pool(name="xtile", bufs=24) as xpool,
            tc.tile_pool(name="pp", bufs=6, space="PSUM") as pp,
            tc.tile_pool(name="vtile", bufs=3) as vpool,
            tc.tile_pool(name="ptr", bufs=2, space="PSUM") as ptr,
        ):
            for tc0 in range(NTC):
                b, jloc = tc0 // NJ, tc0 % NJ
                xts = []
                for kc in range(NCT):
                    xtile = xpool.tile([128, TQ], F32R, tag="x",
                                       name=f"x{tc0}_{kc}")
                    nc.sync.dma_start(xtile[:], xt[kc, tc0])
                    xts.append(xtile)
                for g in range(3):
                    gcol = 128 * g
                    ps = pp.tile([128, TQ], F32, tag="pp", name=f"pp{tc0}_{g}")
                    for kc in range(NCT):
                        nc.tensor.matmul(ps[:], wsb[:, kc, gcol:gcol + 128],
                                         xts[kc][:],
                                         start=(kc == 0), stop=(kc == NCT - 1))
                    chunk = slice(TQ * jloc, TQ * (jloc + 1))
                    if g == 0:        # Q^T of batch b
                        nc.vector.tensor_scalar_add(qts[b][:, chunk], ps[:],
                                                    bsb[:, 0:1])
                    elif g == 1:      # K^T of batch b, split per head
                        nc.vector.tensor_scalar_add(
                            kts[2 * b][0:64, chunk], ps[0:64, :],
                            bsb[0:64, 1:2])
                        nc.vector.tensor_scalar_add(
                            kts[2 * b + 1][64:128, chunk], ps[64:128, :],
                            bsb[64:128, 1:2])
                    else:             # V of batch b -> transpose into VA
                        vtile = vpool.tile([128, TQ], F32R, tag="v",
                                           name=f"v{tc0}")
                        nc.vector.tensor_scalar_add(vtile[:], ps[:],
                                                    bsb[:, 2:3])
                        for q in range(4):
                            tt = jloc * 4 + q   # k-tile index in batch b
                            pst = ptr.tile([128, 128], F32R, tag="pt",
                                           name=f"pt{tc0}_{q}")
                            nc.tensor.matmul(pst[:],
                                             vtile[:, 128 * q:128 * (q + 1)],
                                             ident[:], is_transpose=True)
                            nc.vector.tensor_copy(va[:, b, tt, 0:64],
                                                  pst[:, 0:64])
                            nc.vector.tensor_copy(va[:, b, tt, 128:192],
                                                  pst[:, 64:128])

        if trunc == "proj":
            with tc.tile_pool(name="dumo", bufs=1) as dpool:
                d = dpool.tile([128, TQ], F32, tag="d")
                nc.vector.tensor_copy(d[:], qts[0][0:128, 0:TQ].bitcast(F32))
                nc.sync.dma_start(out[0:128, 0:TQ], d[:])
            return

        # ---- phase 2: attention (p = batch index) ----
        with (
            tc.tile_pool(name="psc", bufs=SPOOL_BUFS, space="PSUM") as spool,
            tc.tile_pool(name="po", bufs=PO_BUFS, space="PSUM") as opool,
            tc.tile_pool(name="ptp", bufs=6) as ptpool,
            tc.tile_pool(name="yt", bufs=3) as ytpool,
            tc.tile_pool(name="rt", bufs=3) as rtpool,
            tc.tile_pool(name="oe", bufs=4) as oepool,
        ):
            order = ([(p, j) for p in range(2) for j in range(NJ)]
                     if not ATTN_INTERLEAVE else
                     [(p, j) for j in range(NJ) for p in range(2)])
            for oi, (p, j) in enumerate(order):
                    if oi == 4 and prefetch_wout == "mid":
                        nc.sync.dma_start(wosb[:],
                                          wout[:].rearrange("(n p) c -> p n c",
                                                            p=128))
                        nc.sync.dma_start(bosb[:], bout[:])
                    nkk = 4 * (j + 1)
                    po = [opool.tile([128, TQ], F32, tag="po",
                                     name=f"po{p}_{j}_{h}") for h in range(2)]
                    for kk in range(nkk):
                        ps_s = spool.tile([128, 2 * TQ], F32, tag="s",
                                          name=f"s{p}_{j}_{kk}")
                        for h2 in range(2):
                            nc.tensor.matmul(
                                ps_s[:, TQ * h2:TQ * (h2 + 1)],
                                kts[2 * p + h2][:, KT * kk:KT * (kk + 1)],
                                qts[p][:, TQ * j:TQ * (j + 1)],
                                start=True, stop=True)
                        pt = ptpool.tile([128, 2 * TQ], F32R, tag="pt",
                                         name=f"p{p}_{j}_{kk}")
                        nc.scalar.activation(pt[:], ps_s[:], AF.Exp)
                        o = max(kk - 4 * j, 0)  # suffix offset (diag tiles)
                        if kk >= 4 * j:
                            for h2 in range(2):
                                lo = TQ * h2 + KT * o
                                # aligned triangle: keep qf' >= r
                                nc.gpsimd.affine_select(
                                    out=pt[:, lo:TQ * (h2 + 1)],
                                    in_=pt[:, lo:TQ * (h2 + 1)],
                                    compare_op=mybir.AluOpType.is_ge,
                                    fill=0.0, base=0,
                                    pattern=[[1, TQ - KT * o]],
                                    channel_multiplier=-1)
                        for h2 in range(2):
                            vs = slice(0, 128) if h2 == 0 else slice(64, 192)
                            nc.tensor.matmul(
                                po[h2][:, KT * o:TQ],
                                va[:, p, kk, vs],
                                pt[:, TQ * h2 + KT * o:TQ * (h2 + 1)],
                                start=(kk == 0), stop=(kk == nkk - 1))
                    # normalize: h0 sums in rows 64:128, h1 sums in rows 0:64
                    yt = ytpool.tile([128, TQ], F32R, tag="yt", name=f"y{p}_{j}")
                    rt = rtpool.tile([128, TQ], F32, tag="rt", name=f"r{p}_{j}")
                    if QUICK_EVICT:
                        # copy psum->sbuf fast so the accumulator banks free
                        # for the next q-chunk before the recip/mul run
                        oes = [oepool.tile([128, TQ], F32, tag="oe",
                                           name=f"oe{p}_{j}_{h}")
                               for h in range(2)]
                        nc.vector.tensor_copy(oes[0][:], po[0][:])
                        nc.vector.tensor_copy(oes[1][:], po[1][:])
                        src0, src1 = oes[0], oes[1]
                    else:
                        src0, src1 = po[0], po[1]
                    nc.vector.reciprocal(rt[0:64, :], src0[64:128, :])
                    nc.vector.tensor_mul(yt[0:64, :], src0[0:64, :], rt[0:64, :])
                    nc.vector.reciprocal(rt[64:128, :], src1[0:64, :])
                    nc.vector.tensor_mul(yt[64:128, :], src1[64:128, :],
                                         rt[64:128, :])
                    nc.sync.dma_start(a2a_in[p * NJ + j, :, :], yt[:])

        if trunc == "attn":
            with tc.tile_pool(name="dumo2", bufs=1) as dpool2:
                d2 = dpool2.tile([128, TQ], F32, tag="d2")
                nc.vector.tensor_copy(d2[:], qts[0][0:128, 0:TQ].bitcast(F32))
                nc.sync.dma_start(out[0:128, 0:TQ], d2[:])
            return

        # ---- phase 3: all-to-all (head-sharded -> t-sharded) ----
        if with_collective is True:
            nc.gpsimd.collective_compute(
                "AllToAll", mybir.AluOpType.bypass,
                replica_groups=[list(range(NCORES))],
                ins=[a2a_in[:]], outs=[a2a_out[:]])
        elif with_collective is False:
            nc.sync.dma_start(a2a_out[:], a2a_in[:])
        # else (None): timing mode — caller aliases a2a_out to a2a_in

        # ---- phase 4: output projection (rows TSL per core) ----
        with (
            tc.tile_pool(name="yts", bufs=1) as ytspool,
            tc.tile_pool(name="pout", bufs=4, space="PSUM") as poutp,
            tc.tile_pool(name="osb", bufs=4) as osbpool,
        ):
            yts = ytspool.tile([128, NCT, TQ], F32R, tag="yts")
            for cc in range(NCT):
                nc.sync.dma_start(yts[:, cc, :], a2a_out[cc, :, :])

            for tt in range(TSL // 128):
                pos = [poutp.tile([128, TQ], F32, tag="pout",
                                  name=f"pos{tt}_{h}") for h in range(2)]
                for cc in range(NCT):
                    for n in range(2):
                        nc.tensor.matmul(
                            pos[n][:], yts[:, cc, 128 * tt:128 * (tt + 1)],
                            wosb[:, cc, TQ * n:TQ * (n + 1)],
                            start=(cc == 0), stop=(cc == NCT - 1))
                for n in range(2):
                    osb = osbpool.tile([128, TQ], F32, tag="osb")
                    nc.vector.tensor_add(osb[:], pos[n][:],
                                         bosb[:, TQ * n:TQ * (n + 1)])
                    nc.sync.dma_start(
                        out[128 * tt:128 * (tt + 1), TQ * n:TQ * (n + 1)],
                        osb[:])


def make_core_inputs(x, w_qkv, b_qkv, w_out, b_out):
    """Host-side shard/transform. Returns list of per-core input dicts."""
    x = np.asarray(x, np.float32)
    w_qkv = np.asarray(w_qkv, np.float32)
    b_qkv = np.asarray(b_qkv, np.float32)
    w_out = np.asarray(w_out, np.float32)
    b_out = np.asarray(b_out, np.float32)

    bout_rep = np.ascontiguousarray(np.broadcast_to(b_out, (128, C)))
    # tile-contiguous x^T: xt[kc, tc0, p, q] = x_flat[TQ*tc0+q, 128*kc+p]
    xt = np.ascontiguousarray(
        x.reshape(NTC, TQ, NCT, 128).transpose(2, 0, 3, 1))
    in_maps = []
    for c in range(NCORES):
        s = slice(128 * c, 128 * (c + 1))
        wq = w_qkv[:, :C][:, s] * 0.125
        wk = w_qkv[:, C:2 * C][:, s]
        wv = w_qkv[:, 2 * C:][:, s]
        wc = np.ascontiguousarray(np.concatenate([wq, wk, wv], axis=1))
        bc3 = np.ascontiguousarray(
            np.stack([b_qkv[:C][s] * 0.125, b_qkv[C:2 * C][s],
                      b_qkv[2 * C:][s]], axis=1))
        in_maps.append({
            "xt": xt, "wqkv": wc, "bqkv": bc3,
            "wout": w_out, "bout": bout_rep,
            "ident": np.eye(128, dtype=np.float32),
            "ones": np.ones((128, 64), np.float32),
            "zeros": np.zeros((64, TQ), np.float32),
        })
    return in_maps


_NC_CACHE = {}


def _make_cached_runner(nc):
    """Jit the SPMD executable once; subsequent calls only re-upload inputs."""
    import jax
    from jax.sharding import Mesh, PartitionSpec
    from jax.experimental.shard_map import shard_map
    from concourse.bass2jax import _bass_exec_p, install_neuronx_cc_hook

    install_neuronx_cc_hook()
    in_names, out_names, out_avals = [], [], []
    for alloc in nc.m.functions[0].allocations:
        if not isinstance(alloc, mybir.MemoryLocationSet):
            continue
        name = alloc.memorylocations[0].name
        if alloc.kind == "ExternalInput":
            in_names.append(name)
        elif alloc.kind == "ExternalOutput":
            out_names.append(name)
            out_avals.append(jax.core.ShapedArray(
                tuple(alloc.tensor_shape), mybir.dt.np(alloc.dtype)))
    n_params = len(in_names)
    all_in = list(in_names) + list(out_names)

    def _body(*args):
        outs = _bass_exec_p.bind(
            *args, out_avals=tuple(out_avals), in_names=tuple(all_in),
            out_names=tuple(out_names), lowering_input_output_aliases=(),
            sim_require_finite=True, sim_require_nnan=True, nc=nc)
        return tuple(outs)

    devices = jax.devices()[:NCORES]
    mesh = Mesh(np.asarray(devices), ("core",))
    spec = PartitionSpec("core")
    sharded = jax.jit(
        shard_map(_body, mesh=mesh,
                  in_specs=(spec,) * (n_params + len(out_names)),
                  out_specs=(spec,) * len(out_names), check_rep=False),
        keep_unused=True)
    zeros = [np.zeros((NCORES * a.shape[0], *a.shape[1:]), a.dtype)
             for a in out_avals]

    def run(in_maps):
        concat = [np.concatenate([np.asarray(m[nm]) for m in in_maps], axis=0)
                  for nm in in_names]
        outs = sharded(*concat, *zeros)
        return {nm: np.asarray(outs[i]) for i, nm in enumerate(out_names)}

    return run


def kernel(x, w_qkv, b_qkv, w_out, b_out):
    in_maps = make_core_inputs(x, w_qkv, b_qkv, w_out, b_out)
    if "nc" not in _NC_CACHE:
        _NC_CACHE["nc"] = build()
    nc = _NC_CACHE["nc"]
    try:
        if "run" not in _NC_CACHE:
            _NC_CACHE["run"] = _make_cached_runner(nc)
        outs = _NC_CACHE["run"](in_maps)
        full = outs["out"].reshape(NCORES * TSL, C)
    except Exception:
        res = run_bass_kernel_spmd(nc, in_maps, core_ids=list(range(NCORES)))
        full = np.concatenate([res.results[c]["out"] for c in range(NCORES)],
                              axis=0)
    return full.reshape(B, T, C)



# revision 2
# speedup vs baseline: 7.7019x; 7.7019x over previous
"""Causal multi-head attention block on 8 TRN2 NeuronCores — v2.

Sharding: tensor-parallel over heads (2 heads/core, both batches) for the
QKV projection + attention; an on-device AllToAll re-shards to
sequence-parallel for the output projection (Megatron-style).

v2 restructure vs baseline:
- x / w_qkv / w_out / attention-output in bf16 (halves input DMA and the
  AllToAll payload); scores + V path stay fp32r.
- batched x DMAs (one per t-chunk), issued before constants; K-padding
  zeros and V-ones come from on-engine memsets, not DMA.
- batch-1 QKV projection is emitted as PE filler inside batch-0
  attention (exp on Act is the attention-phase bottleneck).
- diagonal score tiles only exp the causally-needed column range.
- AllToAll split in two q-halves; output projection accumulates per
  arriving chunk so the second half overlaps compute.

Self-contained: hardcodes all shapes from the problem spec.
"""

import numpy as np
from contextlib import ExitStack

import concourse.bass as bass
import concourse.tile as tile
from concourse import bacc, mybir
from concourse.bass_utils import run_bass_kernel_spmd

F32R = mybir.dt.float32r
F32 = mybir.dt.float32
BF16 = mybir.dt.bfloat16
AF = mybir.ActivationFunctionType

B, T, C, H, HD = 2, 2048, 1024, 16, 64
NCORES = 8
BT = B * T            # 4096 global rows
TQ = 512              # q-chunk width
KT = 128              # k-tile height
NJ = T // TQ          # 4 q-chunks per batch (= per core)
NKK = T // KT         # 16 k-tiles per batch
NCT = C // 128        # 8 contraction tiles for projections
NTC = BT // TQ        # 8 global t-chunks
TSL = BT // NCORES    # 512 rows of final output per core
XT_SHAPE = [NTC, 128, NCT, TQ]  # t-chunk major, partition-major inside


def build(with_collective=True):
    nc = bacc.Bacc(None, target_bir_lowering=False)

    xt = nc.dram_tensor("xt", XT_SHAPE, BF16, kind="ExternalInput")
    wqkv = nc.dram_tensor("wqkv", [C, 3 * 128], BF16, kind="ExternalInput")
    bqkv = nc.dram_tensor("bqkv", [128, 3], F32, kind="ExternalInput")
    wout = nc.dram_tensor("wout", [C, C], BF16, kind="ExternalInput")
    bout = nc.dram_tensor("bout", [128, C], F32, kind="ExternalInput")
    out = nc.dram_tensor("out", [TSL, C], F32, kind="ExternalOutput")

    ident_d = nc.dram_tensor("ident", [128, 128], BF16, kind="ExternalInput")
    mskw_d = nc.dram_tensor("mskw", [128, 4, TQ], BF16, kind="ExternalInput")
    # q-half-major so each half-AllToAll sees a contiguous buffer
    a2a_in = nc.dram_tensor("a2a_in", [2, NCORES, 128, TQ // 2], BF16)
    a2a_out = nc.dram_tensor("a2a_out", [2, NCORES, 128, TQ // 2], BF16)

    with tile.TileContext(nc) as tc:
        _emit(nc, tc, xt, wqkv, bqkv, wout, bout, out, a2a_in, a2a_out,
              ident_d, mskw_d, with_collective)
    nc.compile()
    return nc


def _emit(nc, tc, xt, wqkv, bqkv, wout, bout, out, a2a_in, a2a_out,
          ident_d, mskw_d, with_collective, trunc=None):
    HTQ = TQ // 2  # 256: q-half width for the split A2A

    with ExitStack() as ctx:
        persist = ctx.enter_context(tc.tile_pool(name="persist", bufs=1))

        # persistent SBUF tensors, indexed by batch b (the core owns the
        # same 2 heads in both batches).
        qts = [persist.tile([128, T], F32R, tag=f"qt{p}", name=f"qt{p}")
               for p in range(2)]
        # zero-padded per-head K^T (head h lives in rows 64*(h%2);
        # the other 64 rows are zero so scores run as full K=128 matmuls)
        kts = [persist.tile([128, T], F32R, tag=f"kt{h}", name=f"kt{h}")
               for h in range(4)]
        va = persist.tile([128, 2, NKK, 192], BF16, tag="va")  # [V_e|ones|V_o]
        wsb = persist.tile([128, NCT, 384], BF16, tag="wsb")
        bsb = persist.tile([128, 3], F32, tag="bsb")
        ident = persist.tile([128, 128], BF16, tag="ident")
        wosb = persist.tile([128, NCT, C], BF16, tag="wo")
        bosb = persist.tile([128, C], F32, tag="bo")
        yts = persist.tile([128, NCT, TQ], BF16, tag="yts")
        scr = persist.tile([128, 1], F32, tag="scr")
        mskw = persist.tile([128, 4, TQ], BF16, tag="mskw")

        xpool = ctx.enter_context(tc.tile_pool(name="xtile", bufs=8))
        xsb = {}

        def emit_x_dma(tc0):
            # per-contraction-tile DMAs (128KB each) so transfers spread
            # across DMA engines; one SBUF tile per t-chunk
            xtile = xpool.tile([128, NCT, TQ], BF16, tag="x", name=f"x{tc0}")
            for kc in range(NCT):
                nc.sync.dma_start(xtile[:, kc, :], xt[tc0, :, kc, :])
            xsb[tc0] = xtile

        # weights on the Act DMA queue so they overlap the x stream (SP)
        nc.scalar.dma_start(wsb[:],
                            wqkv[:].rearrange("(n p) c -> p n c", p=128))
        nc.scalar.dma_start(bsb[:], bqkv[:])
        nc.scalar.dma_start(ident[:], ident_d[:])
        nc.scalar.dma_start(mskw[:], mskw_d[:])
        # x stream — PE needs tc0=0 immediately; the rest trickles in
        # under the attention stream
        for tc0 in range(NTC):
            emit_x_dma(tc0)

        # constants via on-engine memsets (no DMA):
        # kts zero padding + the ones block of VA
        for h in range(4):
            dead = slice(64, 128) if h % 2 == 0 else slice(0, 64)
            nc.vector.memset(kts[h][dead, :].bitcast(F32), 0.0)
        nc.vector.memset(va[:, :, :, 64:128], 1.0)

        # warm the Act Exp table off the critical path
        nc.vector.memset(scr[:], 0.0)
        nc.scalar.activation(scr[:], scr[:], AF.Exp)

        vpool = ctx.enter_context(tc.tile_pool(name="vtile", bufs=2))

        def proj_steps(tc0, pp, ptr):
            """Yield the projection of t-chunk tc0 as schedulable steps:
            3 g-steps (8 matmuls + bias add) + 2 transpose-steps."""
            b, jloc = tc0 // NJ, tc0 % NJ
            chunk = slice(TQ * jloc, TQ * (jloc + 1))
            vtile = [None]

            def g_step(g):
                gcol = 128 * g
                ps = pp.tile([128, TQ], F32, tag="pp", name=f"pp{tc0}_{g}")
                for kc in range(NCT):
                    nc.tensor.matmul(ps[:], wsb[:, kc, gcol:gcol + 128],
                                     xsb[tc0][:, kc, :],
                                     start=(kc == 0), stop=(kc == NCT - 1))
                if g == 0:        # Q^T of batch b
                    nc.vector.tensor_scalar_add(qts[b][:, chunk], ps[:],
                                                bsb[:, 0:1])
                elif g == 1:      # K^T of batch b, split per head
                    nc.vector.tensor_scalar_add(
                        kts[2 * b][0:64, chunk], ps[0:64, :], bsb[0:64, 1:2])
                    nc.vector.tensor_scalar_add(
                        kts[2 * b + 1][64:128, chunk], ps[64:128, :],
                        bsb[64:128, 1:2])
                else:             # V of batch b
                    vt = vpool.tile([128, TQ], BF16, tag="v", name=f"v{tc0}")
                    nc.vector.tensor_scalar_add(vt[:], ps[:], bsb[:, 2:3])
                    vtile[0] = vt

            def t_step(qpair):
                for q in (2 * qpair, 2 * qpair + 1):
                    tt = jloc * 4 + q   # k-tile index in batch b
                    pst = ptr.tile([128, 128], BF16, tag="pt",
                                   name=f"pt{tc0}_{q}")
                    nc.tensor.matmul(pst[:], vtile[0][:, 128 * q:128 * (q + 1)],
                                     ident[:], is_transpose=True)
                    nc.vector.tensor_copy(va[:, b, tt, 0:64], pst[:, 0:64])
                    nc.vector.tensor_copy(va[:, b, tt, 128:192],
                                          pst[:, 64:128])

            yield from (lambda g=g: g_step(g) for g in range(3))
            yield from (lambda qp=qp: t_step(qp) for qp in range(2))

        # ---- prefix: projection of t-chunk 0 only ----
        with (
            tc.tile_pool(name="pp_pre", bufs=3, space="PSUM") as pp_pre,
            tc.tile_pool(name="ptr_pre", bufs=2, space="PSUM") as ptr_pre,
        ):
            for step in proj_steps(0, pp_pre, ptr_pre):
                step()
            if trunc == "proj":
                for tc0 in range(1, 8):
                    for step in proj_steps(tc0, pp_pre, ptr_pre):
                        step()
        if trunc == "proj":
            with tc.tile_pool(name="dum", bufs=1) as dpool:
                d = dpool.tile([128, TQ], F32, tag="d")
                nc.vector.tensor_copy(d[:], qts[0][:, 0:TQ].bitcast(F32))
                nc.sync.dma_start(out[0:128, 0:TQ], d[:])
            return

        # ---- attention (p = batch index), filler-interleaved ----
        with (
            tc.tile_pool(name="pp", bufs=1, space="PSUM") as pp,
            tc.tile_pool(name="ptr", bufs=1, space="PSUM") as ptr,
            tc.tile_pool(name="psc", bufs=2, space="PSUM") as spool,
            tc.tile_pool(name="po", bufs=2, space="PSUM") as opool,
            tc.tile_pool(name="ptp", bufs=6) as ptpool,
            tc.tile_pool(name="yt", bufs=3) as ytpool,
            tc.tile_pool(name="rt", bufs=3) as rtpool,
            tc.tile_pool(name="oe", bufs=4) as oepool,
        ):
            # filler: projection of t-chunks 1..7, interleaved into the
            # attention stream (chunk c occupies filler steps 5(c-1)..5c)
            filler = []
            for tc0 in range(1, 8):
                filler.extend(proj_steps(tc0, pp, ptr))

            total_kk = 2 * sum(4 * (j + 1) for j in range(NJ))  # 80
            nfill = len(filler)
            fill_state = {"done": 0, "kk": 0}
            po_t = {}

            def fill_until(n):
                n = min(n, nfill)
                while fill_state["done"] < n:
                    filler[fill_state["done"]]()
                    fill_state["done"] += 1

            def emit_scores(p, j, kk):
                """Score matmuls + exp for one k-tile. Diagonal tiles get
                the causal mask added in-PSUM by an extra accumulating
                matmul (ident^T @ mskw = -30 on the masked triangle), so
                nothing sits between the exp and the AV matmul."""
                ps_s = spool.tile([128, 2 * TQ], F32, tag="s",
                                  name=f"s{p}_{j}_{kk}")
                diag = kk >= 4 * j
                o = max(kk - 4 * j, 0)  # suffix offset (diag tiles)
                for h2 in range(2):
                    nc.tensor.matmul(
                        ps_s[:, TQ * h2:TQ * (h2 + 1)],
                        kts[2 * p + h2][:, KT * kk:KT * (kk + 1)],
                        qts[p][:, TQ * j:TQ * (j + 1)],
                        start=True, stop=not diag)
                    if diag:
                        lo = TQ * h2 + KT * o
                        nc.tensor.matmul(
                            ps_s[:, lo:TQ * (h2 + 1)],
                            ident[:], mskw[:, o, KT * o:],
                            start=False, stop=True)
                pt = ptpool.tile([128, 2 * TQ], BF16, tag="pt",
                                 name=f"p{p}_{j}_{kk}")
                if o == 0:
                    nc.scalar.activation(pt[:], ps_s[:], AF.Exp)
                else:
                    # diagonal tiles: only exp the causal column range
                    for h2 in range(2):
                        lo = TQ * h2 + KT * o
                        nc.scalar.activation(pt[:, lo:TQ * (h2 + 1)],
                                             ps_s[:, lo:TQ * (h2 + 1)],
                                             AF.Exp)
                return pt

            def emit_av(p, j, kk, pt):
                """Accumulate one k-tile into the (p, j) output."""
                nkk = 4 * (j + 1)
                if kk == 0:
                    po_t[(p, j)] = [
                        opool.tile([128, TQ], F32, tag="po",
                                   name=f"po{p}_{j}_{h}") for h in range(2)]
                po = po_t[(p, j)]
                o = max(kk - 4 * j, 0)
                for h2 in range(2):
                    vs = slice(0, 128) if h2 == 0 else slice(64, 192)
                    nc.tensor.matmul(
                        po[h2][:, KT * o:TQ],
                        va[:, p, kk, vs],
                        pt[:, TQ * h2 + KT * o:TQ * (h2 + 1)],
                        start=(kk == 0), stop=(kk == nkk - 1))

            def emit_norm(p, j):
                """Normalize the finished (p, j) chunk and ship it.
                h0 sums sit in rows 64:128, h1 sums in rows 0:64."""
                po = po_t.pop((p, j))
                yt = ytpool.tile([128, TQ], BF16, tag="yt", name=f"y{p}_{j}")
                rt = rtpool.tile([128, TQ], F32, tag="rt", name=f"r{p}_{j}")
                # copy psum->sbuf fast so the accumulator banks free
                # for the next q-chunk before the recip/mul run
                oes = [oepool.tile([128, TQ], F32, tag="oe",
                                   name=f"oe{p}_{j}_{h}") for h in range(2)]
                nc.vector.tensor_copy(oes[0][:], po[0][:])
                nc.vector.tensor_copy(oes[1][:], po[1][:])
                nc.vector.reciprocal(rt[0:64, :], oes[0][64:128, :])
                nc.vector.tensor_mul(yt[0:64, :], oes[0][0:64, :], rt[0:64, :])
                nc.vector.reciprocal(rt[64:128, :], oes[1][0:64, :])
                nc.vector.tensor_mul(yt[64:128, :], oes[1][64:128, :],
                                     rt[64:128, :])
                for hv in range(2):
                    nc.sync.dma_start(a2a_in[hv, p * NJ + j, :, :],
                                      yt[:, HTQ * hv:HTQ * (hv + 1)])

            def emit_fill():
                fill_state["kk"] += 1
                fill_until((fill_state["kk"] * nfill) // total_kk)

            # flattened (p, j, kk) stream, software-pipelined with
            # lookahead 1: AV(k-1) is emitted after S(k)/exp(k) so the PE
            # queue never sits on the exp latency; projection filler rides
            # between S(k) and AV(k-1). The (p, j) block reads chunks
            # <= 4p+j, so that chunk's projection is forced out first.
            steps = [(p, j, kk) for p in range(2) for j in range(NJ)
                     for kk in range(4 * (j + 1))]
            prev = None
            for p, j, kk in steps:
                if kk == 0:
                    fill_until(5 * (4 * p + j))
                pt = emit_scores(p, j, kk)
                emit_fill()
                if prev is not None:
                    emit_av(*prev)
                    pp_, jp_, kkp_, _ = prev
                    if kkp_ == 4 * (jp_ + 1) - 1:
                        emit_norm(pp_, jp_)
                if p == 0 and j == 0 and kk == 2:
                    # w_out prefetch on the Act queue, issued once the
                    # startup DMA burst has drained
                    nc.scalar.dma_start(
                        wosb[:], wout[:].rearrange("(n p) c -> p n c", p=128))
                    nc.scalar.dma_start(bosb[:], bout[:])
                prev = (p, j, kk, pt)
            emit_av(*prev)
            emit_norm(prev[0], prev[1])
            fill_until(nfill)   # safety: shouldn't trigger

        if trunc == "attn":
            with tc.tile_pool(name="dum2", bufs=1) as dpool2:
                d2 = dpool2.tile([128, TQ], F32, tag="d2")
                nc.vector.tensor_copy(d2[:], qts[0][:, 0:TQ].bitcast(F32))
                nc.sync.dma_start(out[0:128, 0:TQ], d2[:])
            return

        # ---- all-to-all (head-sharded -> t-sharded), split in q-halves ----
        for hv in range(2):
            if with_collective is True:
                nc.gpsimd.collective_compute(
                    "AllToAll", mybir.AluOpType.bypass,
                    replica_groups=[list(range(NCORES))],
                    ins=[a2a_in[hv]], outs=[a2a_out[hv]])
            elif with_collective is False:
                nc.sync.dma_start(a2a_out[hv], a2a_in[hv])
            # else (None): timing mode — caller aliases a2a_out to a2a_in

        # ---- output projection (rows TSL per core), chunk-streamed ----
        poutp = ctx.enter_context(
            tc.tile_pool(name="pout", bufs=4, space="PSUM"))
        osbpool = ctx.enter_context(tc.tile_pool(name="osb", bufs=4))

        pos = {}
        for hv in range(2):
            qsl = slice(HTQ * hv, HTQ * (hv + 1))
            tts = (0, 1) if hv == 0 else (2, 3)
            for cc in range(NCT):
                nc.scalar.dma_start(yts[:, cc, qsl], a2a_out[hv, cc, :, :])
            for tt in tts:
                pos[tt] = poutp.tile([128, C], F32, tag="pout",
                                     name=f"pos{tt}")
            for cc in range(NCT):
                for tt in tts:
                    for n in range(2):
                        nc.tensor.matmul(
                            pos[tt][:, TQ * n:TQ * (n + 1)],
                            yts[:, cc, 128 * tt:128 * (tt + 1)],
                            wosb[:, cc, TQ * n:TQ * (n + 1)],
                            start=(cc == 0), stop=(cc == NCT - 1))
            for tt in tts:
                for n in range(2):
                    osb = osbpool.tile([128, TQ], F32, tag="osb")
                    nc.vector.tensor_add(osb[:], pos[tt][:, TQ * n:TQ * (n + 1)],
                                         bosb[:, TQ * n:TQ * (n + 1)])
                    nc.sync.dma_start(
                        out[128 * tt:128 * (tt + 1), TQ * n:TQ * (n + 1)],
                        osb[:])


def make_core_inputs(x, w_qkv, b_qkv, w_out, b_out):
    """Host-side shard/transform. Returns list of per-core input dicts."""
    import ml_dtypes
    bf16 = ml_dtypes.bfloat16

    x = np.asarray(x, np.float32)
    w_qkv = np.asarray(w_qkv, np.float32)
    b_qkv = np.asarray(b_qkv, np.float32)
    w_out = np.asarray(w_out, np.float32)
    b_out = np.asarray(b_out, np.float32)

    bout_rep = np.ascontiguousarray(np.broadcast_to(b_out, (128, C)))
    # x^T tiles: xt[tc0, p, kc, q] = x_flat[TQ*tc0+q, 128*kc+p]
    xt = np.ascontiguousarray(
        x.reshape(NTC, TQ, NCT, 128).transpose(0, 3, 2, 1)).astype(bf16)
    wout_bf = w_out.astype(bf16)
    in_maps = []
    for c in range(NCORES):
        s = slice(128 * c, 128 * (c + 1))
        wq = w_qkv[:, :C][:, s] * 0.125
        wk = w_qkv[:, C:2 * C][:, s]
        wv = w_qkv[:, 2 * C:][:, s]
        wc = np.ascontiguousarray(
            np.concatenate([wq, wk, wv], axis=1)).astype(bf16)
        bc3 = np.ascontiguousarray(
            np.stack([b_qkv[:C][s] * 0.125, b_qkv[C:2 * C][s],
                      b_qkv[2 * C:][s]], axis=1))
        in_maps.append({
            "xt": xt, "wqkv": wc, "bqkv": bc3,
            "wout": wout_bf, "bout": bout_rep,
            "ident": np.eye(128, dtype=np.float32).astype(bf16),
            "mskw": np.stack(
                [np.where(np.arange(TQ)[None, :]
                          < np.arange(128)[:, None] + 128 * o,
                          -30.0, 0.0).astype(np.float32)
                 for o in range(4)], axis=1).astype(bf16),
        })
    return in_maps


_NC_CACHE = {}


def _make_cached_runner(nc):
    """Jit the SPMD executable once; subsequent calls only re-upload inputs."""
    import jax
    from jax.sharding import Mesh, PartitionSpec
    from jax.experimental.shard_map import shard_map
    from concourse.bass2jax import (_bass_exec_p, install_neuronx_cc_hook,
                                    partition_id_tensor)

    install_neuronx_cc_hook()
    partition_name = (nc.partition_id_tensor.name
                      if nc.partition_id_tensor else None)
    in_names, out_names, out_avals = [], [], []
    for alloc in nc.m.functions[0].allocations:
        if not isinstance(alloc, mybir.MemoryLocationSet):
            continue
        name = alloc.memorylocations[0].name
        if alloc.kind == "ExternalInput":
            if name != partition_name:
                in_names.append(name)
        elif alloc.kind == "ExternalOutput":
            out_names.append(name)
            out_avals.append(jax.core.ShapedArray(
                tuple(alloc.tensor_shape), mybir.dt.np(alloc.dtype)))
    n_params = len(in_names)
    all_in = list(in_names) + list(out_names)
    if partition_name is not None:
        all_in.append(partition_name)

    def _body(*args):
        operands = list(args)
        if partition_name is not None:
            operands.append(partition_id_tensor())
        outs = _bass_exec_p.bind(
            *operands, out_avals=tuple(out_avals), in_names=tuple(all_in),
            out_names=tuple(out_names), lowering_input_output_aliases=(),
            sim_require_finite=True, sim_require_nnan=True, nc=nc)
        return tuple(outs)

    devices = jax.devices()[:NCORES]
    mesh = Mesh(np.asarray(devices), ("core",))
    spec = PartitionSpec("core")
    sharded = jax.jit(
        shard_map(_body, mesh=mesh,
                  in_specs=(spec,) * (n_params + len(out_names)),
                  out_specs=(spec,) * len(out_names), check_rep=False),
        keep_unused=True)
    zeros = [np.zeros((NCORES * a.shape[0], *a.shape[1:]), a.dtype)
             for a in out_avals]

    def run(in_maps):
        concat = [np.concatenate([np.asarray(m[nm]) for m in in_maps], axis=0)
                  for nm in in_names]
        outs = sharded(*concat, *zeros)
        return {nm: np.asarray(outs[i]) for i, nm in enumerate(out_names)}

    return run


def kernel(x, w_qkv, b_qkv, w_out, b_out):
    in_maps = make_core_inputs(x, w_qkv, b_qkv, w_out, b_out)
    if "nc" not in _NC_CACHE:
        _NC_CACHE["nc"] = build()
    nc = _NC_CACHE["nc"]
    try:
        if "run" not in _NC_CACHE:
            _NC_CACHE["run"] = _make_cached_runner(nc)
        outs = _NC_CACHE["run"](in_maps)
        full = outs["out"].reshape(NCORES * TSL, C)
    except Exception:
        res = run_bass_kernel_spmd(nc, in_maps, core_ids=list(range(NCORES)))
        full = np.concatenate([res.results[c]["out"] for c in range(NCORES)],
                              axis=0)
    return full.reshape(B, T, C)
